# revision 1
# baseline (speedup 1.0000x reference)
"""Trainium2 Bass kernel for nn_Embedding_61366492725854.

Computes einsum('bsi,ie->bse', inputs, embedding) with
B,S,I,E = 64,4096,128,128 — i.e. a (262144,128)@(128,128) f32 matmul.

Strategy (memory-bound, data-parallel over 8 NeuronCores):
  - Flatten inputs to (B*S, I), shard rows evenly: 32768 rows/core.
  - Per core, stream row-groups of 2048 rows (1 MiB DMAs):
      DMA in -> PE transpose (128x128 tiles, via identity matmul) -> PSUM
      -> VectorE copy PSUM->SBUF -> PE matmul (X_t^T stationary, W moving)
      -> PSUM -> ScalarE copy PSUM->SBUF -> DMA out.
  - In-DMAs issued from SP (sync), out-DMAs from ACT: two separate
    HWDGE rings so reads and writes overlap.
"""

import numpy as np

from concourse import bacc, bass, mybir
from concourse import tile
from concourse import bass_utils

B, S, I, E = 64, 4096, 128, 128
N_CORES = 8
ROWS = B * S                 # 262144
R = ROWS // N_CORES          # 32768 rows per core
GROUP = 2048                 # rows per DMA group (1 MiB f32)
JT = GROUP // 128            # 16 row-tiles per group
NG = R // GROUP              # 16 groups per core
SUB = 8                      # row-tiles per PSUM sub-group (2 banks)
NSUB = JT // SUB             # 2 sub-groups per group

F32 = mybir.dt.float32


def _build_nc():
    nc = bacc.Bacc(
        "TRN2",
        target_bir_lowering=False,
        debug=False,
        enable_asserts=False,
        num_devices=N_CORES,
    )
    x = nc.dram_tensor("x", [R, I], F32, kind="ExternalInput")
    w = nc.dram_tensor("w", [I, E], F32, kind="ExternalInput")
    ident = nc.dram_tensor("ident", [128, 128], F32, kind="ExternalInput")
    out = nc.dram_tensor("out", [R, E], F32, kind="ExternalOutput")

    # [g, p, j, i]: group, partition(row%128), row-tile in group, feature
    xg = x.ap().rearrange("(g j p) i -> g p j i", p=128, j=JT)
    og = out.ap().rearrange("(g j p) e -> g p j e", p=128, j=JT)

    with tile.TileContext(nc) as tc:
        with (
            tc.tile_pool(name="consts", bufs=1) as consts,
            tc.tile_pool(name="xin", bufs=3) as xin,
            tc.tile_pool(name="xt", bufs=3) as xtp,
            tc.tile_pool(name="outp", bufs=3) as outp,
            tc.tile_pool(name="ps_xt", bufs=2, space=bass.MemorySpace.PSUM) as psxt,
            tc.tile_pool(name="ps_o", bufs=2, space=bass.MemorySpace.PSUM) as pso,
        ):
            w_t = consts.tile([I, E], F32)
            id_t = consts.tile([128, 128], F32)
            nc.sync.dma_start(w_t[:], w.ap())
            nc.sync.dma_start(id_t[:], ident.ap())

            for g in range(NG):
                x_t = xin.tile([128, JT, 128], F32)
                nc.sync.dma_start(x_t[:], xg[g])
                o_t = outp.tile([128, JT, 128], F32)
                for sgi in range(NSUB):
                    ps_xt = psxt.tile([128, SUB, 128], F32)
                    for j in range(SUB):
                        jj = sgi * SUB + j
                        nc.tensor.transpose(ps_xt[:, j, :], x_t[:, jj, :], id_t[:])
                    xt_t = xtp.tile([128, SUB, 128], F32)
                    nc.vector.tensor_copy(xt_t[:], ps_xt[:])
                    ps_o = pso.tile([128, SUB, 128], F32)
                    for j in range(SUB):
                        nc.tensor.matmul(
                            ps_o[:, j, :], xt_t[:, j, :], w_t[:],
                            start=True, stop=True,
                        )
                    nc.scalar.copy(o_t[:, sgi * SUB:(sgi + 1) * SUB, :], ps_o[:])
                nc.scalar.dma_start(og[g], o_t[:])

    nc.compile()
    return nc


_cached_nc = None


def _run(X, W, trace=False, trace_kwargs=None):
    """X: (ROWS, I) f32, W: (I, E) f32 -> (ROWS, E) f32 (+ results obj)."""
    global _cached_nc
    if _cached_nc is None:
        _cached_nc = _build_nc()
    nc = _cached_nc
    ident = np.eye(128, dtype=np.float32)
    in_maps = [
        {"x": X[c * R:(c + 1) * R], "w": W, "ident": ident}
        for c in range(N_CORES)
    ]
    res = bass_utils.run_bass_kernel_spmd(
        nc, in_maps, core_ids=list(range(N_CORES)),
        trace=trace, **(trace_kwargs or {}),
    )
    outs = np.concatenate(
        [res.results[c]["out"] for c in range(N_CORES)], axis=0
    )
    return outs, res


def kernel(inputs, embedding):
    X = np.ascontiguousarray(np.asarray(inputs, dtype=np.float32)).reshape(ROWS, I)
    W = np.ascontiguousarray(np.asarray(embedding, dtype=np.float32))
    outs, _ = _run(X, W)
    return outs.reshape(B, S, E)


# revision 2
# speedup vs baseline: 1.0216x; 1.0216x over previous
"""Trainium2 Bass kernel for nn_Embedding_61366492725854.

Computes einsum('bsi,ie->bse', inputs, embedding) with
B,S,I,E = 64,4096,128,128 — i.e. a (262144,128)@(128,128) f32 matmul.

Strategy (memory-bound, data-parallel over 8 NeuronCores):
  - Flatten inputs to (B*S, I), shard rows evenly: 32768 rows/core.
  - Per core, stream row-groups of 2048 rows (1 MiB DMAs):
      DMA in -> PE transpose (128x128 tiles, via identity matmul) -> PSUM
      -> VectorE copy PSUM->SBUF -> PE matmul (X_t^T stationary, W moving)
      -> PSUM -> ScalarE copy PSUM->SBUF -> DMA out.
  - In-DMAs issued from SP (sync), out-DMAs from ACT: two separate
    HWDGE rings so reads and writes overlap.
"""

import numpy as np

from concourse import bacc, bass, mybir
from concourse import tile
from concourse import bass_utils

B, S, I, E = 64, 4096, 128, 128
N_CORES = 8
ROWS = B * S                 # 262144
R = ROWS // N_CORES          # 32768 rows per core
GROUP = 2048                 # rows per DMA group (1 MiB f32)
JT = GROUP // 128            # 16 row-tiles per group
NG = R // GROUP              # 16 groups per core
SUB = 8                      # row-tiles per PSUM sub-group (2 banks)
NSUB = JT // SUB             # 2 sub-groups per group

F32 = mybir.dt.float32


def _build_nc():
    nc = bacc.Bacc(
        "TRN2",
        target_bir_lowering=False,
        debug=False,
        enable_asserts=False,
        num_devices=N_CORES,
    )
    x = nc.dram_tensor("x", [R, I], F32, kind="ExternalInput")
    w = nc.dram_tensor("w", [I, E], F32, kind="ExternalInput")
    ident = nc.dram_tensor("ident", [128, 128], F32, kind="ExternalInput")
    out = nc.dram_tensor("out", [R, E], F32, kind="ExternalOutput")

    # Row r = g*GROUP + p*JT + k lives at tile[p, k, :] — each partition
    # line covers JT *consecutive* DRAM rows (JT*512B contiguous per
    # descriptor). The same permutation is used on input and output, and
    # the matmul is row-independent, so the result is still row-exact.
    xg = x.ap().rearrange("(g p k) i -> g p k i", p=128, k=JT)
    og = out.ap().rearrange("(g p k) e -> g p k e", p=128, k=JT)

    with tile.TileContext(nc) as tc:
        with (
            tc.tile_pool(name="consts", bufs=1) as consts,
            tc.tile_pool(name="xin", bufs=3) as xin,
            tc.tile_pool(name="xt", bufs=3) as xtp,
            tc.tile_pool(name="outp", bufs=3) as outp,
            tc.tile_pool(name="ps_xt", bufs=2, space=bass.MemorySpace.PSUM) as psxt,
            tc.tile_pool(name="ps_o", bufs=2, space=bass.MemorySpace.PSUM) as pso,
        ):
            w_t = consts.tile([I, E], F32)
            id_t = consts.tile([128, 128], F32)
            nc.sync.dma_start(w_t[:], w.ap())
            nc.sync.dma_start(id_t[:], ident.ap())

            for g in range(NG):
                x_t = xin.tile([128, JT, 128], F32)
                nc.sync.dma_start(x_t[:], xg[g])
                o_t = outp.tile([128, JT, 128], F32)
                for sgi in range(NSUB):
                    ps_xt = psxt.tile([128, SUB, 128], F32)
                    for j in range(SUB):
                        jj = sgi * SUB + j
                        nc.tensor.transpose(ps_xt[:, j, :], x_t[:, jj, :], id_t[:])
                    xt_t = xtp.tile([128, SUB, 128], F32)
                    nc.vector.tensor_copy(xt_t[:], ps_xt[:])
                    ps_o = pso.tile([128, SUB, 128], F32)
                    for j in range(SUB):
                        nc.tensor.matmul(
                            ps_o[:, j, :], xt_t[:, j, :], w_t[:],
                            start=True, stop=True,
                        )
                    nc.scalar.copy(o_t[:, sgi * SUB:(sgi + 1) * SUB, :], ps_o[:])
                nc.scalar.dma_start(og[g], o_t[:])

    nc.compile()
    return nc


_cached_nc = None


def _run(X, W, trace=False, trace_kwargs=None):
    """X: (ROWS, I) f32, W: (I, E) f32 -> (ROWS, E) f32 (+ results obj)."""
    global _cached_nc
    if _cached_nc is None:
        _cached_nc = _build_nc()
    nc = _cached_nc
    ident = np.eye(128, dtype=np.float32)
    in_maps = [
        {"x": X[c * R:(c + 1) * R], "w": W, "ident": ident}
        for c in range(N_CORES)
    ]
    res = bass_utils.run_bass_kernel_spmd(
        nc, in_maps, core_ids=list(range(N_CORES)),
        trace=trace, **(trace_kwargs or {}),
    )
    outs = np.concatenate(
        [res.results[c]["out"] for c in range(N_CORES)], axis=0
    )
    return outs, res


def kernel(inputs, embedding):
    X = np.ascontiguousarray(np.asarray(inputs, dtype=np.float32)).reshape(ROWS, I)
    W = np.ascontiguousarray(np.asarray(embedding, dtype=np.float32))
    outs, _ = _run(X, W)
    return outs.reshape(B, S, E)
